# revision 24
# baseline (speedup 1.0000x reference)
"""Trainium2 Bass kernel for nn_AdvOneLayer (dense_mlp, memory-bound).

Math (see the PyTorch/JAX reference):
    W1_norm[j] = sum_i |W1[j, i]|                       # [H]
    pert[b,i,j] = -eps * y[b,i] * sign(W2[i,j]) * W1_norm[j]   # [B, O, H]
    nn_output[b,i] = H * sum_j W2[i,j] + bias2[i]       # [B, O], independent of b

Sharding: H (=4096) is split 512-per-core across 8 NeuronCores.  Each core
reads only its W1/W2 slice (2MB + 0.5MB) plus the replicated y (64KB) and
writes its 32MB slab of pert.  The tiny nn_output reduction over j is done
as per-core partials that are summed on the host during the gather step
(the "all-reduce on the sum over j" from the sharding hint, realized at
unshard time).

Per-core dataflow:
  - W1 slice [512,1024] -> 4 SBUF tiles [128,1024]; DVE abs-reduce ->
    norm_col4[p,t] (partition-major norms).
  - PE transpose [128,4]->[4,128], then 4 rank-1 matmuls (ones x norm_row)
    broadcast the norms to all partitions: bnorm[q, j] = norm[j].
  - C[h] = sign(W2_h) * bnorm   for the two 128-row halves of O=256.
  - A[p, t] = -eps * yT[p, t]  where t indexes the 128 row-tiles of the
    flattened (b,i) dim: flat = t*128 + p, i = (t%2)*128 + p.
  - 128 output tiles: pert_tile[p, :] = A[p, t] * C[t%2][p, :], staged in
    4MB chunks (16 tiles) and written with one large DMA each.
"""

import sys

sys.path.insert(0, "/opt/trn_rl_repo")

import numpy as np

import concourse.bass as bass
import concourse.tile as tile
from concourse import masks, mybir
from concourse.bass_utils import run_bass_kernel_spmd
from concourse.vector_clock import ScopedClock


def _patched_drain_and_barrier(self, tick_clock, wait_clock):
    """Replacement for TileContext._drain_and_barrier: the walrus codegen in
    this toolchain allows only a limited number of sync waits per instruction,
    so spread the end-of-kernel drain's waits over consecutive single-wait
    drain instructions instead of attaching all of them to one."""
    drain_inst = self.nc.sync.drain()
    wait_clock.add_sem_waits(
        drain_inst.ins, ScopedClock({None: tick_clock.global_clock})
    )
    si = drain_inst.ins.sync_info
    if si is not None and si.on_wait and len(si.on_wait) > 1:
        waits = list(si.on_wait)
        si.on_wait = waits[:1]
        for w in waits[1:]:
            extra = self.nc.sync.drain()
            extra.ins.sync_info = mybir.SyncInfo(on_wait=[w], on_update=[])

    self.nc.all_engine_barrier()
    assert self.sems is not None
    popped = self.nc._tile_sem_poison_stack.pop()
    assert popped is self._sem_poison
    self.nc.clear_and_free_semaphores(list(self.sems.allocated().values()))
    self.nc.all_engine_barrier()


tile.TileContext._drain_and_barrier = _patched_drain_and_barrier

EPS = 0.1
B, I, O, H = 64, 1024, 256, 4096
NCORES = 8
HS = H // NCORES            # 512 hidden units per core
T = (B * O) // 128          # 128 row-tiles of the flattened (b,i) dim
G = 16                      # tiles per store chunk -> 16*128*512*4 = 4 MB DMA

_NC_CACHE = None


def _build_kernel():
    nc = bass.Bass()
    f32 = mybir.dt.float32
    w1s = nc.declare_dram_parameter("w1s", [HS, I], f32, isOutput=False)
    w2s = nc.declare_dram_parameter("w2s", [O, HS], f32, isOutput=False)
    yt = nc.declare_dram_parameter("yt", [128, T], f32, isOutput=False)
    pert_out = nc.declare_dram_parameter("pert_out", [T, 128, HS], f32, isOutput=True)
    s_out = nc.declare_dram_parameter("s_out", [128, 2], f32, isOutput=True)

    # Leading store chunks are small so the first store issues as early as
    # possible; steady-state chunks are 16 tiles per DMA.
    chunk_plan = [2, 4, 10] + [G] * ((T - 16) // G)
    assert sum(chunk_plan) == T
    # NJ=2 was tried (store j-block 0 while block 1's norms are computed) but
    # loses: halving the TSP width doubles the DVE op count and the per-op
    # read-write bubble makes DVE the bottleneck (81us vs 62us busy).
    NJ = 1
    SUB = HS // NJ
    SEG = 128         # C is built per 128-column segment (transpose width)

    with tile.TileContext(nc, pool_alloc_mode="queue") as tc:
        with (
            tc.tile_pool(name="singles", bufs=1) as singles,
            tc.tile_pool(name="work", bufs=2) as work,
            tc.tile_pool(name="stage", bufs=3) as stagep,
            tc.tile_pool(name="psum", bufs=1, space="PSUM") as psum,
        ):
            # ones vector for the rank-1 broadcast matmul (DVE so the matmul's
            # deps funnel through a single semaphore)
            ones1 = singles.tile([1, 128], f32)
            nc.vector.memset(ones1[:], 1.0)
            identity = singles.tile([128, 128], f32)
            masks.make_identity(nc, identity[:])
            # identity "observer": a dummy DVE op waiting on the gpsimd-built
            # identity.  Every later DVE tick then transitively implies the
            # identity is ready, so the PE transposes only need their DVE
            # (norm) wait and the Pool wait can be stripped.
            id_obs = singles.tile([1, 1], f32)
            nc.vector.tensor_copy(id_obs[:], identity[0:1, 0:1])

            # ---- loads: W1 in pipelined chunks on the SP ring; W2 + y on
            # the ACT ring so they land while W1 streams ----------------------
            w2_sb = singles.tile([128, 2, HS], f32)
            nc.scalar.dma_start(out=w2_sb[:], in_=w2s.rearrange("(h p) j -> p h j", p=128))
            yt_sb = singles.tile([128, T], f32)
            nc.scalar.dma_start(out=yt_sb[:], in_=yt[:, :])
            w1_sb = singles.tile([128, 4, I], f32)
            w1v = w1s.rearrange("(t p) i -> p t i", p=128)
            for t4 in range(3):
                nc.sync.dma_start(out=w1_sb[:, t4, :], in_=w1v[:, t4, :])
            # last chunk split in two so its reduce starts half a load earlier
            nc.sync.dma_start(out=w1_sb[:, 3, 0 : I // 2], in_=w1v[:, 3, 0 : I // 2])
            nc.sync.dma_start(out=w1_sb[:, 3, I // 2 : I], in_=w1v[:, 3, I // 2 : I])

            # ---- sign chain on gpsimd (otherwise idle) while W1 streams -----
            sgn = singles.tile([128, 2, HS], f32)
            for h in range(2):
                gt_t = work.tile([128, HS], f32)
                nc.gpsimd.tensor_scalar(
                    out=gt_t[:], in0=w2_sb[:, h, :], scalar1=0.0, scalar2=None,
                    op0=mybir.AluOpType.is_gt,
                )
                lt_t = work.tile([128, HS], f32)
                nc.gpsimd.tensor_scalar(
                    out=lt_t[:], in0=w2_sb[:, h, :], scalar1=0.0, scalar2=None,
                    op0=mybir.AluOpType.is_lt,
                )
                nc.gpsimd.tensor_sub(sgn[:, h, :], gt_t[:], lt_t[:])
            # sign observer on DVE: later DVE ticks imply sgn is complete, so
            # each C-segment TT needs only its PE (broadcast) wait
            sgn_obs = singles.tile([1, 1], f32)
            nc.vector.tensor_copy(sgn_obs[:], sgn[0:1, 1, HS - 1 : HS])

            # ---- A[p, t] = -eps * yT[p, t] ----------------------------------
            a_sb = singles.tile([128, T], f32)
            nc.vector.tensor_scalar_mul(a_sb[:], yt_sb[:], -EPS)

            # ---- W1 row 1-norms, fully pipelined with the chunk loads:
            # reduce (DVE) -> column transpose into a [1, 512] PSUM row at
            # partition 0 -> copy segment to SBUF -> rank-1 broadcast (PE) ->
            # C segments (DVE) ------------------------------------------------
            norm_col4 = singles.tile([128, 6], f32)
            normrow_ps = psum.tile([1, HS], f32)
            norm_row = singles.tile([1, HS], f32)
            bnorm_ps = psum.tile([128, HS], f32)
            c_sb = singles.tile([128, 2, HS], f32)
            for t4 in range(4):
                if t4 < 3:
                    nc.vector.tensor_reduce(
                        out=norm_col4[:, t4 : t4 + 1],
                        in_=w1_sb[:, t4, :],
                        axis=mybir.AxisListType.X,
                        op=mybir.AluOpType.add,
                        apply_absolute_value=True,
                    )
                else:
                    # last chunk was loaded in halves
                    for hh in range(2):
                        nc.vector.tensor_reduce(
                            out=norm_col4[:, 4 + hh : 5 + hh],
                            in_=w1_sb[:, 3, hh * (I // 2) : (hh + 1) * (I // 2)],
                            axis=mybir.AxisListType.X,
                            op=mybir.AluOpType.add,
                            apply_absolute_value=True,
                        )
                    nc.vector.tensor_add(
                        norm_col4[:, 3:4], norm_col4[:, 4:5], norm_col4[:, 5:6]
                    )
                seg = slice(t4 * SEG, (t4 + 1) * SEG)
                nc.tensor.transpose(
                    normrow_ps[0:1, seg], norm_col4[:, t4 : t4 + 1], identity[:]
                )
                nc.vector.tensor_copy(norm_row[0:1, seg], normrow_ps[0:1, seg])
                nc.tensor.matmul(bnorm_ps[:, seg], ones1[:], norm_row[0:1, seg])
                for h in range(2):
                    nc.vector.tensor_mul(
                        c_sb[:, h, seg], sgn[:, h, seg], bnorm_ps[:, seg]
                    )

            # ---- main loop: pert row-tiles, j-block 0 first -----------------
            last_tsp = None
            for b in range(NJ):
                jlo, jhi = b * SUB, (b + 1) * SUB
                t = 0
                for g_chunk in chunk_plan:
                    stage = stagep.tile([128, G, SUB], f32)
                    t0 = t
                    for g in range(g_chunk):
                        last_tsp = nc.vector.tensor_scalar_mul(
                            out=stage[:, g, :],
                            in0=c_sb[:, t % 2, jlo:jhi],
                            scalar1=a_sb[:, t : t + 1],
                        )
                        t += 1
                    nc.sync.dma_start(
                        out=pert_out[t0:t, :, jlo:jhi].rearrange("t p j -> p t j"),
                        in_=stage[:, 0:g_chunk, :],
                    )

            # ---- per-core partial row sums of W2 (for nn_output) ------------
            # Pinned after the main loop (explicit dep, since the scheduler
            # orders by readiness): the DVE tick gating this store then
            # transitively implies all earlier DMA completions, so its HWDGE
            # lane-reuse wait is provably dead and can be stripped.
            s_sb = singles.tile([128, 2], f32)
            for h in range(2):
                red = nc.vector.reduce_sum(
                    out=s_sb[:, h : h + 1],
                    in_=w2_sb[:, h, :],
                    axis=mybir.AxisListType.X,
                )
                tile.add_dep_helper(
                    red.ins, last_tsp.ins, sync=False,
                    reason="keep s-reduces after the pert main loop",
                )
            nc.sync.dma_start(out=s_out[:, :], in_=s_sb[:])
    return nc


def _strip_redundant_waits(nc):
    """Reduce every instruction to (at most) one semaphore wait when provably
    safe.  The walrus codegen in this toolchain supports only ONE sync wait
    per compute/DMA instruction, but Tile's semaphore assignment freely emits
    two (e.g. a data dep plus a DMA-lane-reuse dep that is transitively
    implied by the data dep).

    Soundness model: engines execute their instruction streams in order and a
    compute instruction's own-engine semaphore increments fire at completion
    before the next same-engine instruction dispatches.  DMA-lane semaphores
    (DMAHW*/DMASW*) increment at *transfer* completion, so they contribute
    guarantees only through explicit waits, never through program order.

    We propagate, per instruction X, a map G[X]: sem -> value guaranteed to
    have been reached when X dispatches:
      - same-engine predecessor's guarantees + its waits + its own-engine
        increments (serial completion),
      - for each wait (S, v) where S is an engine-owned semaphore, the full
        guarantee set of the producer instruction that raised S to v.
    A wait (S, v) on X is dead if the guarantees from X's *other* waits plus
    program order already imply S >= v.
    """
    fn = nc.m.functions[0]

    insts = []
    for bb in fn.blocks:
        insts.extend(bb.instructions)

    def sem_waits(inst):
        si = getattr(inst, "sync_info", None)
        if not si or not si.on_wait:
            return []
        return [
            w
            for w in si.on_wait
            if w.sync_type == "semaphore" and "ge" in str(w.wait_mode)
        ]

    # Identify engine-owned semaphores: updated exclusively by instructions of
    # one engine AND not a DMA-completion semaphore (those fire out of band).
    upd_engines = {}
    for inst in insts:
        si = getattr(inst, "sync_info", None)
        if si:
            for u in si.on_update or []:
                upd_engines.setdefault(u.id, set()).add(str(inst.engine))
    engine_sem_owner = {}
    for inst in insts:
        si = getattr(inst, "sync_info", None)
        if si:
            for u in si.on_update or []:
                name = u.ant_name or ""
                if name.startswith(("DMAHW", "DMASW")):
                    continue
                if upd_engines.get(u.id) and len(upd_engines[u.id]) == 1:
                    engine_sem_owner[u.id] = next(iter(upd_engines[u.id]))

    # Forward pass: cumulative own-engine increments and guarantee maps.
    def merge(dst, src):
        for k, v in src.items():
            if dst.get(k, -1) < v:
                dst[k] = v

    cum = {}  # sem_id -> cumulative inc
    producer_g = {}  # sem_id -> list of (cum_value_after, guarantee map)
    g_complete_prev = {}  # engine -> guarantee map after its last instruction
    g_dispatch = [None] * len(insts)

    def closure_of_wait(sem_id, value):
        # A wait (S, v) guarantees S >= v, plus everything the producer that
        # raised S to v was itself guaranteed.  For engine sems the producer's
        # completion-time guarantees apply; for DMA sems the transfer fires
        # after the DMACopy's dispatch, so its dispatch-time guarantees (incl.
        # its own waits) apply.
        out = {sem_id: value}
        hist = producer_g.get(sem_id, [])
        for cum_v, g in hist:
            if cum_v >= value:
                merge(out, g)
                break
        return out

    n_drop = 0
    for idx, inst in enumerate(insts):
        eng = str(inst.engine)
        g = dict(g_complete_prev.get(eng, {}))
        waits = sem_waits(inst)
        # prune: keep the minimal wait set (greedy: try each wait as the sole
        # keeper and check whether it implies all the others)
        si = getattr(inst, "sync_info", None)
        if si is not None and si.on_wait and len(si.on_wait) > 1 and waits:
            all_waits = list(si.on_wait)
            best = None
            for keeper in all_waits:
                ctx = dict(g)
                merge(ctx, closure_of_wait(keeper.id, keeper.wait_value or 0))
                if all(
                    (w is keeper)
                    or (ctx.get(w.id, -1) >= (w.wait_value or 0))
                    for w in all_waits
                ):
                    best = keeper
                    break
            if best is not None:
                n_drop += len(all_waits) - 1
                si.on_wait = [best]
                waits = [best]
            else:
                # keep all; compile may still reject, but don't guess
                pass
        # fold this instruction's (kept) waits into its dispatch guarantees
        for w in waits:
            merge(g, closure_of_wait(w.id, w.wait_value or 0))
        g_dispatch[idx] = g
        # completion: own-engine sem increments become guaranteed for the
        # next same-engine instruction; DMA-sem increments fire later (at
        # transfer completion) but still carry this dispatch's guarantees.
        g_after = dict(g)
        if si:
            for u in si.on_update or []:
                name = u.ant_name or ""
                if engine_sem_owner.get(u.id) == eng:
                    cum[u.id] = cum.get(u.id, 0) + (u.update_value or 0)
                    g_after[u.id] = max(g_after.get(u.id, -1), cum[u.id])
                    producer_g.setdefault(u.id, []).append((cum[u.id], g_after))
                elif name.startswith(("DMAHW", "DMASW")):
                    cum[u.id] = cum.get(u.id, 0) + (u.update_value or 0)
                    producer_g.setdefault(u.id, []).append((cum[u.id], dict(g)))
        g_complete_prev[eng] = g_after
    return n_drop


def _get_nc():
    global _NC_CACHE
    if _NC_CACHE is None:
        nc = _build_kernel()
        _strip_redundant_waits(nc)
        _NC_CACHE = nc
    return _NC_CACHE


def _run(inputs, trace=False):
    y = np.asarray(inputs["y"])
    W1 = np.asarray(inputs["W1"], dtype=np.float32)
    W2 = np.asarray(inputs["W2"], dtype=np.float32)
    bias2 = np.asarray(inputs["bias2"], dtype=np.float32)

    # yT[p, t] = y.flat[t*128 + p]  (so column t is the per-partition scalar
    # vector for row-tile t of the flattened (b, i) dim)
    yT = np.ascontiguousarray(y.astype(np.float32).reshape(T, 128).T)

    in_maps = []
    for c in range(NCORES):
        in_maps.append(
            {
                "w1s": np.ascontiguousarray(W1[c * HS : (c + 1) * HS, :]),
                "w2s": np.ascontiguousarray(W2[:, c * HS : (c + 1) * HS]),
                "yt": yT,
            }
        )

    nc = _get_nc()
    bres = run_bass_kernel_spmd(nc, in_maps, core_ids=list(range(NCORES)), trace=trace)
    res = bres.results

    pert = np.empty((B, O, H), dtype=np.float32)
    s_total = np.zeros(O, dtype=np.float32)
    for c in range(NCORES):
        pert[:, :, c * HS : (c + 1) * HS] = res[c]["pert_out"].reshape(B, O, HS)
        # s_out[p, h] = sum_j W2s[h*128+p, j]  ->  index i = h*128 + p
        s_total += res[c]["s_out"].T.reshape(O)

    nn_row = (np.float32(H) * s_total + bias2).astype(np.float32)
    nn_output = np.broadcast_to(nn_row, (B, O)).copy()
    return (nn_output, pert), bres


def kernel(**inputs):
    (nn_output, pert), _ = _run(inputs, trace=False)
    return nn_output, pert


# revision 25
# speedup vs baseline: 1.2441x; 1.2441x over previous
"""Trainium2 Bass kernel for nn_AdvOneLayer (dense_mlp, memory-bound).

Math (see the PyTorch/JAX reference):
    W1_norm[j] = sum_i |W1[j, i]|                       # [H]
    pert[b,i,j] = -eps * y[b,i] * sign(W2[i,j]) * W1_norm[j]   # [B, O, H]
    nn_output[b,i] = H * sum_j W2[i,j] + bias2[i]       # [B, O], independent of b

Sharding: H (=4096) is split 512-per-core across 8 NeuronCores.  Each core
reads only its W1/W2 slice (2MB + 0.5MB) plus the replicated y (64KB) and
writes its 32MB slab of pert.  The tiny nn_output reduction over j is done
as per-core partials that are summed on the host during the gather step
(the "all-reduce on the sum over j" from the sharding hint, realized at
unshard time).

Per-core dataflow:
  - W1 slice [512,1024] -> 4 SBUF tiles [128,1024]; DVE abs-reduce ->
    norm_col4[p,t] (partition-major norms).
  - PE transpose [128,4]->[4,128], then 4 rank-1 matmuls (ones x norm_row)
    broadcast the norms to all partitions: bnorm[q, j] = norm[j].
  - C[h] = sign(W2_h) * bnorm   for the two 128-row halves of O=256.
  - A[p, t] = -eps * yT[p, t]  where t indexes the 128 row-tiles of the
    flattened (b,i) dim: flat = t*128 + p, i = (t%2)*128 + p.
  - 128 output tiles: pert_tile[p, :] = A[p, t] * C[t%2][p, :], staged in
    4MB chunks (16 tiles) and written with one large DMA each.
"""

import sys

sys.path.insert(0, "/opt/trn_rl_repo")

import numpy as np

import concourse.bass as bass
import concourse.tile as tile
from concourse import masks, mybir
from concourse.bass_utils import run_bass_kernel_spmd
from concourse.vector_clock import ScopedClock


def _patched_drain_and_barrier(self, tick_clock, wait_clock):
    """Replacement for TileContext._drain_and_barrier: the walrus codegen in
    this toolchain allows only a limited number of sync waits per instruction,
    so spread the end-of-kernel drain's waits over consecutive single-wait
    drain instructions instead of attaching all of them to one."""
    drain_inst = self.nc.sync.drain()
    wait_clock.add_sem_waits(
        drain_inst.ins, ScopedClock({None: tick_clock.global_clock})
    )
    si = drain_inst.ins.sync_info
    if si is not None and si.on_wait and len(si.on_wait) > 1:
        waits = list(si.on_wait)
        si.on_wait = waits[:1]
        for w in waits[1:]:
            extra = self.nc.sync.drain()
            extra.ins.sync_info = mybir.SyncInfo(on_wait=[w], on_update=[])

    self.nc.all_engine_barrier()
    assert self.sems is not None
    popped = self.nc._tile_sem_poison_stack.pop()
    assert popped is self._sem_poison
    self.nc.clear_and_free_semaphores(list(self.sems.allocated().values()))
    self.nc.all_engine_barrier()


tile.TileContext._drain_and_barrier = _patched_drain_and_barrier

EPS = 0.1
B, I, O, H = 64, 1024, 256, 4096
NCORES = 8
HS = H // NCORES            # 512 hidden units per core
T = (B * O) // 128          # 128 row-tiles of the flattened (b,i) dim
G = 16                      # tiles per store chunk -> 16*128*512*4 = 4 MB DMA

_NC_CACHE = None


def _build_kernel():
    nc = bass.Bass()
    f32 = mybir.dt.float32
    w1s = nc.declare_dram_parameter("w1s", [HS, I], f32, isOutput=False)
    w2s = nc.declare_dram_parameter("w2s", [O, HS], f32, isOutput=False)
    yt = nc.declare_dram_parameter("yt", [128, T], f32, isOutput=False)
    pert_out = nc.declare_dram_parameter("pert_out", [T, 128, HS], f32, isOutput=True)
    s_out = nc.declare_dram_parameter("s_out", [128, 2], f32, isOutput=True)

    # Leading store chunks are small so the first store issues as early as
    # possible; steady-state chunks are 16 tiles per DMA.
    chunk_plan = [2, 4, 10] + [G] * ((T - 16) // G)
    assert sum(chunk_plan) == T
    # NJ=2 was tried (store j-block 0 while block 1's norms are computed) but
    # loses: halving the TSP width doubles the DVE op count and the per-op
    # read-write bubble makes DVE the bottleneck (81us vs 62us busy).
    NJ = 1
    SUB = HS // NJ
    SEG = 128         # C is built per 128-column segment (transpose width)

    with tile.TileContext(nc, pool_alloc_mode="queue") as tc:
        with (
            tc.tile_pool(name="singles", bufs=1) as singles,
            tc.tile_pool(name="work", bufs=2) as work,
            tc.tile_pool(name="stage", bufs=3) as stagep,
            tc.tile_pool(name="psum", bufs=1, space="PSUM") as psum,
        ):
            # ones vector for the rank-1 broadcast matmul (DVE so the matmul's
            # deps funnel through a single semaphore)
            ones1 = singles.tile([1, 128], f32)
            nc.vector.memset(ones1[:], 1.0)
            identity = singles.tile([128, 128], f32)
            masks.make_identity(nc, identity[:])
            # identity "observer": a dummy DVE op waiting on the gpsimd-built
            # identity.  Every later DVE tick then transitively implies the
            # identity is ready, so the PE transposes only need their DVE
            # (norm) wait and the Pool wait can be stripped.
            id_obs = singles.tile([1, 1], f32)
            nc.vector.tensor_copy(id_obs[:], identity[0:1, 0:1])

            # ---- loads: W1 in pipelined chunks on the SP ring; W2 + y on
            # the ACT ring so they land while W1 streams ----------------------
            w2_sb = singles.tile([128, 2, HS], f32)
            nc.scalar.dma_start(out=w2_sb[:], in_=w2s.rearrange("(h p) j -> p h j", p=128))
            yt_sb = singles.tile([128, T], f32)
            nc.scalar.dma_start(out=yt_sb[:], in_=yt[:, :])
            w1_sb = singles.tile([128, 4, I], f32)
            w1v = w1s.rearrange("(t p) i -> p t i", p=128)
            for t4 in range(3):
                nc.sync.dma_start(out=w1_sb[:, t4, :], in_=w1v[:, t4, :])
            # last chunk split in two so its reduce starts half a load earlier
            nc.sync.dma_start(out=w1_sb[:, 3, 0 : I // 2], in_=w1v[:, 3, 0 : I // 2])
            nc.sync.dma_start(out=w1_sb[:, 3, I // 2 : I], in_=w1v[:, 3, I // 2 : I])

            # ---- sign chain on DVE (gpsimd is ~20x too slow for elementwise,
            # ACT would add a second wait to every C op) -----------------------
            sgn = singles.tile([128, 2, HS], f32)
            for h in range(2):
                gt_t = work.tile([128, HS], f32)
                nc.vector.tensor_scalar(
                    out=gt_t[:], in0=w2_sb[:, h, :], scalar1=0.0, scalar2=None,
                    op0=mybir.AluOpType.is_gt,
                )
                lt_t = work.tile([128, HS], f32)
                nc.vector.tensor_scalar(
                    out=lt_t[:], in0=w2_sb[:, h, :], scalar1=0.0, scalar2=None,
                    op0=mybir.AluOpType.is_lt,
                )
                nc.vector.tensor_sub(sgn[:, h, :], gt_t[:], lt_t[:])

            # ---- A[p, t] = -eps * yT[p, t] ----------------------------------
            a_sb = singles.tile([128, T], f32)
            nc.vector.tensor_scalar_mul(a_sb[:], yt_sb[:], -EPS)

            # ---- W1 row 1-norms, fully pipelined with the chunk loads:
            # reduce (DVE) -> column transpose into a [1, 512] PSUM row at
            # partition 0 -> copy segment to SBUF -> rank-1 broadcast (PE) ->
            # C segments (DVE) ------------------------------------------------
            norm_col4 = singles.tile([128, 6], f32)
            normrow_ps = psum.tile([1, HS], f32)
            norm_row = singles.tile([1, HS], f32)
            bnorm_ps = psum.tile([128, HS], f32)
            c_sb = singles.tile([128, 2, HS], f32)
            for t4 in range(4):
                if t4 < 3:
                    nc.vector.tensor_reduce(
                        out=norm_col4[:, t4 : t4 + 1],
                        in_=w1_sb[:, t4, :],
                        axis=mybir.AxisListType.X,
                        op=mybir.AluOpType.add,
                        apply_absolute_value=True,
                    )
                else:
                    # last chunk was loaded in halves
                    for hh in range(2):
                        nc.vector.tensor_reduce(
                            out=norm_col4[:, 4 + hh : 5 + hh],
                            in_=w1_sb[:, 3, hh * (I // 2) : (hh + 1) * (I // 2)],
                            axis=mybir.AxisListType.X,
                            op=mybir.AluOpType.add,
                            apply_absolute_value=True,
                        )
                    nc.vector.tensor_add(
                        norm_col4[:, 3:4], norm_col4[:, 4:5], norm_col4[:, 5:6]
                    )
                seg = slice(t4 * SEG, (t4 + 1) * SEG)
                nc.tensor.transpose(
                    normrow_ps[0:1, seg], norm_col4[:, t4 : t4 + 1], identity[:]
                )
                nc.vector.tensor_copy(norm_row[0:1, seg], normrow_ps[0:1, seg])
                nc.tensor.matmul(bnorm_ps[:, seg], ones1[:], norm_row[0:1, seg])
                for h in range(2):
                    nc.vector.tensor_mul(
                        c_sb[:, h, seg], sgn[:, h, seg], bnorm_ps[:, seg]
                    )

            # ---- main loop: pert row-tiles, j-block 0 first -----------------
            last_tsp = None
            for b in range(NJ):
                jlo, jhi = b * SUB, (b + 1) * SUB
                t = 0
                for g_chunk in chunk_plan:
                    stage = stagep.tile([128, G, SUB], f32)
                    t0 = t
                    for g in range(g_chunk):
                        last_tsp = nc.vector.tensor_scalar_mul(
                            out=stage[:, g, :],
                            in0=c_sb[:, t % 2, jlo:jhi],
                            scalar1=a_sb[:, t : t + 1],
                        )
                        t += 1
                    nc.sync.dma_start(
                        out=pert_out[t0:t, :, jlo:jhi].rearrange("t p j -> p t j"),
                        in_=stage[:, 0:g_chunk, :],
                    )

            # ---- per-core partial row sums of W2 (for nn_output) ------------
            # Pinned after the main loop (explicit dep, since the scheduler
            # orders by readiness): the DVE tick gating this store then
            # transitively implies all earlier DMA completions, so its HWDGE
            # lane-reuse wait is provably dead and can be stripped.
            s_sb = singles.tile([128, 2], f32)
            for h in range(2):
                red = nc.vector.reduce_sum(
                    out=s_sb[:, h : h + 1],
                    in_=w2_sb[:, h, :],
                    axis=mybir.AxisListType.X,
                )
                tile.add_dep_helper(
                    red.ins, last_tsp.ins, sync=False,
                    reason="keep s-reduces after the pert main loop",
                )
            nc.sync.dma_start(out=s_out[:, :], in_=s_sb[:])
    return nc


def _strip_redundant_waits(nc):
    """Reduce every instruction to (at most) one semaphore wait when provably
    safe.  The walrus codegen in this toolchain supports only ONE sync wait
    per compute/DMA instruction, but Tile's semaphore assignment freely emits
    two (e.g. a data dep plus a DMA-lane-reuse dep that is transitively
    implied by the data dep).

    Soundness model: engines execute their instruction streams in order and a
    compute instruction's own-engine semaphore increments fire at completion
    before the next same-engine instruction dispatches.  DMA-lane semaphores
    (DMAHW*/DMASW*) increment at *transfer* completion, so they contribute
    guarantees only through explicit waits, never through program order.

    We propagate, per instruction X, a map G[X]: sem -> value guaranteed to
    have been reached when X dispatches:
      - same-engine predecessor's guarantees + its waits + its own-engine
        increments (serial completion),
      - for each wait (S, v) where S is an engine-owned semaphore, the full
        guarantee set of the producer instruction that raised S to v.
    A wait (S, v) on X is dead if the guarantees from X's *other* waits plus
    program order already imply S >= v.
    """
    fn = nc.m.functions[0]

    insts = []
    for bb in fn.blocks:
        insts.extend(bb.instructions)

    def sem_waits(inst):
        si = getattr(inst, "sync_info", None)
        if not si or not si.on_wait:
            return []
        return [
            w
            for w in si.on_wait
            if w.sync_type == "semaphore" and "ge" in str(w.wait_mode)
        ]

    # Identify engine-owned semaphores: updated exclusively by instructions of
    # one engine AND not a DMA-completion semaphore (those fire out of band).
    upd_engines = {}
    for inst in insts:
        si = getattr(inst, "sync_info", None)
        if si:
            for u in si.on_update or []:
                upd_engines.setdefault(u.id, set()).add(str(inst.engine))
    engine_sem_owner = {}
    for inst in insts:
        si = getattr(inst, "sync_info", None)
        if si:
            for u in si.on_update or []:
                name = u.ant_name or ""
                if name.startswith(("DMAHW", "DMASW")):
                    continue
                if upd_engines.get(u.id) and len(upd_engines[u.id]) == 1:
                    engine_sem_owner[u.id] = next(iter(upd_engines[u.id]))

    # Forward pass: cumulative own-engine increments and guarantee maps.
    def merge(dst, src):
        for k, v in src.items():
            if dst.get(k, -1) < v:
                dst[k] = v

    cum = {}  # sem_id -> cumulative inc
    producer_g = {}  # sem_id -> list of (cum_value_after, guarantee map)
    g_complete_prev = {}  # engine -> guarantee map after its last instruction
    g_dispatch = [None] * len(insts)

    def closure_of_wait(sem_id, value):
        # A wait (S, v) guarantees S >= v, plus everything the producer that
        # raised S to v was itself guaranteed.  For engine sems the producer's
        # completion-time guarantees apply; for DMA sems the transfer fires
        # after the DMACopy's dispatch, so its dispatch-time guarantees (incl.
        # its own waits) apply.
        out = {sem_id: value}
        hist = producer_g.get(sem_id, [])
        for cum_v, g in hist:
            if cum_v >= value:
                merge(out, g)
                break
        return out

    n_drop = 0
    for idx, inst in enumerate(insts):
        eng = str(inst.engine)
        g = dict(g_complete_prev.get(eng, {}))
        waits = sem_waits(inst)
        # prune: keep the minimal wait set (greedy: try each wait as the sole
        # keeper and check whether it implies all the others)
        si = getattr(inst, "sync_info", None)
        if si is not None and si.on_wait and len(si.on_wait) > 1 and waits:
            all_waits = list(si.on_wait)
            best = None
            for keeper in all_waits:
                ctx = dict(g)
                merge(ctx, closure_of_wait(keeper.id, keeper.wait_value or 0))
                if all(
                    (w is keeper)
                    or (ctx.get(w.id, -1) >= (w.wait_value or 0))
                    for w in all_waits
                ):
                    best = keeper
                    break
            if best is not None:
                n_drop += len(all_waits) - 1
                si.on_wait = [best]
                waits = [best]
            else:
                # keep all; compile may still reject, but don't guess
                pass
        # fold this instruction's (kept) waits into its dispatch guarantees
        for w in waits:
            merge(g, closure_of_wait(w.id, w.wait_value or 0))
        g_dispatch[idx] = g
        # completion: own-engine sem increments become guaranteed for the
        # next same-engine instruction; DMA-sem increments fire later (at
        # transfer completion) but still carry this dispatch's guarantees.
        g_after = dict(g)
        if si:
            for u in si.on_update or []:
                name = u.ant_name or ""
                if engine_sem_owner.get(u.id) == eng:
                    cum[u.id] = cum.get(u.id, 0) + (u.update_value or 0)
                    g_after[u.id] = max(g_after.get(u.id, -1), cum[u.id])
                    producer_g.setdefault(u.id, []).append((cum[u.id], g_after))
                elif name.startswith(("DMAHW", "DMASW")):
                    cum[u.id] = cum.get(u.id, 0) + (u.update_value or 0)
                    producer_g.setdefault(u.id, []).append((cum[u.id], dict(g)))
        g_complete_prev[eng] = g_after
    return n_drop


def _get_nc():
    global _NC_CACHE
    if _NC_CACHE is None:
        nc = _build_kernel()
        _strip_redundant_waits(nc)
        _NC_CACHE = nc
    return _NC_CACHE


def _run(inputs, trace=False):
    y = np.asarray(inputs["y"])
    W1 = np.asarray(inputs["W1"], dtype=np.float32)
    W2 = np.asarray(inputs["W2"], dtype=np.float32)
    bias2 = np.asarray(inputs["bias2"], dtype=np.float32)

    # yT[p, t] = y.flat[t*128 + p]  (so column t is the per-partition scalar
    # vector for row-tile t of the flattened (b, i) dim)
    yT = np.ascontiguousarray(y.astype(np.float32).reshape(T, 128).T)

    in_maps = []
    for c in range(NCORES):
        in_maps.append(
            {
                "w1s": np.ascontiguousarray(W1[c * HS : (c + 1) * HS, :]),
                "w2s": np.ascontiguousarray(W2[:, c * HS : (c + 1) * HS]),
                "yt": yT,
            }
        )

    nc = _get_nc()
    bres = run_bass_kernel_spmd(nc, in_maps, core_ids=list(range(NCORES)), trace=trace)
    res = bres.results

    pert = np.empty((B, O, H), dtype=np.float32)
    s_total = np.zeros(O, dtype=np.float32)
    for c in range(NCORES):
        pert[:, :, c * HS : (c + 1) * HS] = res[c]["pert_out"].reshape(B, O, HS)
        # s_out[p, h] = sum_j W2s[h*128+p, j]  ->  index i = h*128 + p
        s_total += res[c]["s_out"].T.reshape(O)

    nn_row = (np.float32(H) * s_total + bias2).astype(np.float32)
    nn_output = np.broadcast_to(nn_row, (B, O)).copy()
    return (nn_output, pert), bres


def kernel(**inputs):
    (nn_output, pert), _ = _run(inputs, trace=False)
    return nn_output, pert


# revision 27
# speedup vs baseline: 1.3287x; 1.0680x over previous
"""Trainium2 Bass kernel for nn_AdvOneLayer (dense_mlp, memory-bound).

Math (see the PyTorch/JAX reference):
    W1_norm[j] = sum_i |W1[j, i]|                       # [H]
    pert[b,i,j] = -eps * y[b,i] * sign(W2[i,j]) * W1_norm[j]   # [B, O, H]
    nn_output[b,i] = H * sum_j W2[i,j] + bias2[i]       # [B, O], independent of b

Sharding: H (=4096) is split 512-per-core across 8 NeuronCores.  Each core
reads only its W1/W2 slice (2MB + 0.5MB) plus the replicated y (64KB) and
writes its 32MB slab of pert.  The tiny nn_output reduction over j is done
as per-core partials that are summed on the host during the gather step
(the "all-reduce on the sum over j" from the sharding hint, realized at
unshard time).

Per-core dataflow:
  - W1 slice [512,1024] -> 4 SBUF tiles [128,1024]; DVE abs-reduce ->
    norm_col4[p,t] (partition-major norms).
  - PE transpose [128,4]->[4,128], then 4 rank-1 matmuls (ones x norm_row)
    broadcast the norms to all partitions: bnorm[q, j] = norm[j].
  - C[h] = sign(W2_h) * bnorm   for the two 128-row halves of O=256.
  - A[p, t] = -eps * yT[p, t]  where t indexes the 128 row-tiles of the
    flattened (b,i) dim: flat = t*128 + p, i = (t%2)*128 + p.
  - 128 output tiles: pert_tile[p, :] = A[p, t] * C[t%2][p, :], staged in
    4MB chunks (16 tiles) and written with one large DMA each.
"""

import sys

sys.path.insert(0, "/opt/trn_rl_repo")

import numpy as np

import concourse.bass as bass
import concourse.tile as tile
from concourse import masks, mybir
from concourse.bass_utils import run_bass_kernel_spmd
from concourse.vector_clock import ScopedClock


def _patched_drain_and_barrier(self, tick_clock, wait_clock):
    """Replacement for TileContext._drain_and_barrier: the walrus codegen in
    this toolchain allows only a limited number of sync waits per instruction,
    so spread the end-of-kernel drain's waits over consecutive single-wait
    drain instructions instead of attaching all of them to one."""
    drain_inst = self.nc.sync.drain()
    wait_clock.add_sem_waits(
        drain_inst.ins, ScopedClock({None: tick_clock.global_clock})
    )
    si = drain_inst.ins.sync_info
    if si is not None and si.on_wait and len(si.on_wait) > 1:
        waits = list(si.on_wait)
        si.on_wait = waits[:1]
        for w in waits[1:]:
            extra = self.nc.sync.drain()
            extra.ins.sync_info = mybir.SyncInfo(on_wait=[w], on_update=[])

    self.nc.all_engine_barrier()
    assert self.sems is not None
    popped = self.nc._tile_sem_poison_stack.pop()
    assert popped is self._sem_poison
    self.nc.clear_and_free_semaphores(list(self.sems.allocated().values()))
    self.nc.all_engine_barrier()


tile.TileContext._drain_and_barrier = _patched_drain_and_barrier

EPS = 0.1
B, I, O, H = 64, 1024, 256, 4096
NCORES = 8
HS = H // NCORES            # 512 hidden units per core
T = (B * O) // 128          # 128 row-tiles of the flattened (b,i) dim
G = 16                      # tiles per store chunk -> 16*128*512*4 = 4 MB DMA

_NC_CACHE = None


def _build_kernel():
    nc = bass.Bass()
    f32 = mybir.dt.float32
    w1s = nc.declare_dram_parameter("w1s", [HS, I], f32, isOutput=False)
    w2s = nc.declare_dram_parameter("w2s", [O, HS], f32, isOutput=False)
    yt = nc.declare_dram_parameter("yt", [128, T], f32, isOutput=False)
    pert_out = nc.declare_dram_parameter("pert_out", [T, 128, HS], f32, isOutput=True)
    s_out = nc.declare_dram_parameter("s_out", [128, 2], f32, isOutput=True)

    # Leading store chunks are small so the first store issues as early as
    # possible; steady-state chunks are 16 tiles per DMA.
    chunk_plan = [2, 2, 4, 8] + [G] * ((T - 16) // G)
    assert sum(chunk_plan) == T
    # NJ=2 was tried (store j-block 0 while block 1's norms are computed) but
    # loses: halving the TSP width doubles the DVE op count and the per-op
    # read-write bubble makes DVE the bottleneck (81us vs 62us busy).
    NJ = 1
    SUB = HS // NJ
    SEG = 128         # C is built per 128-column segment (transpose width)

    with tile.TileContext(nc, pool_alloc_mode="queue") as tc:
        with (
            tc.tile_pool(name="singles", bufs=1) as singles,
            tc.tile_pool(name="work", bufs=2) as work,
            tc.tile_pool(name="stage", bufs=3) as stagep,
            tc.tile_pool(name="psum", bufs=1, space="PSUM") as psum,
        ):
            # ones vector for the rank-1 broadcast matmul (DVE so the matmul's
            # deps funnel through a single semaphore)
            ones1 = singles.tile([1, 128], f32)
            nc.vector.memset(ones1[:], 1.0)
            identity = singles.tile([128, 128], f32)
            masks.make_identity(nc, identity[:])
            # identity "observer": a dummy DVE op waiting on the gpsimd-built
            # identity.  Every later DVE tick then transitively implies the
            # identity is ready, so the PE transposes only need their DVE
            # (norm) wait and the Pool wait can be stripped.
            id_obs = singles.tile([1, 1], f32)
            nc.vector.tensor_copy(id_obs[:], identity[0:1, 0:1])

            # ---- loads: W1 in pipelined chunks on the SP ring; W2 + y on
            # the ACT ring so they land while W1 streams ----------------------
            w2_sb = singles.tile([128, 2, HS], f32)
            nc.scalar.dma_start(out=w2_sb[:], in_=w2s.rearrange("(h p) j -> p h j", p=128))
            yt_sb = singles.tile([128, T], f32)
            nc.scalar.dma_start(out=yt_sb[:], in_=yt[:, :])
            w1_sb = singles.tile([128, 4, I], f32)
            w1v = w1s.rearrange("(t p) i -> p t i", p=128)
            for t4 in range(3):
                nc.sync.dma_start(out=w1_sb[:, t4, :], in_=w1v[:, t4, :])
            # last chunk split in two so its reduce starts half a load earlier
            nc.sync.dma_start(out=w1_sb[:, 3, 0 : I // 2], in_=w1v[:, 3, 0 : I // 2])
            nc.sync.dma_start(out=w1_sb[:, 3, I // 2 : I], in_=w1v[:, 3, I // 2 : I])

            # ---- W1 row 1-norms, fully pipelined with the chunk loads:
            # reduce (DVE) -> column transpose into a [1, 512] PSUM row at
            # partition 0 -> copy segment to SBUF -> rank-1 broadcast (PE) ->
            # C segments (DVE) ------------------------------------------------
            norm_col4 = singles.tile([128, 6], f32)
            normrow_ps = psum.tile([1, HS], f32)
            norm_row = singles.tile([1, HS], f32)
            bnorm_ps = psum.tile([128, HS], f32)
            c_sb = singles.tile([128, 2, HS], f32)
            for t4 in range(4):
                if t4 < 3:
                    nc.vector.tensor_reduce(
                        out=norm_col4[:, t4 : t4 + 1],
                        in_=w1_sb[:, t4, :],
                        axis=mybir.AxisListType.X,
                        op=mybir.AluOpType.add,
                        apply_absolute_value=True,
                    )
                else:
                    # last chunk was loaded in halves
                    for hh in range(2):
                        nc.vector.tensor_reduce(
                            out=norm_col4[:, 4 + hh : 5 + hh],
                            in_=w1_sb[:, 3, hh * (I // 2) : (hh + 1) * (I // 2)],
                            axis=mybir.AxisListType.X,
                            op=mybir.AluOpType.add,
                            apply_absolute_value=True,
                        )
                    nc.vector.tensor_add(
                        norm_col4[:, 3:4], norm_col4[:, 4:5], norm_col4[:, 5:6]
                    )
                seg = slice(t4 * SEG, (t4 + 1) * SEG)
                nc.tensor.transpose(
                    normrow_ps[0:1, seg], norm_col4[:, t4 : t4 + 1], identity[:]
                )
                nc.vector.tensor_copy(norm_row[0:1, seg], normrow_ps[0:1, seg])
                nc.tensor.matmul(bnorm_ps[:, seg], ones1[:], norm_row[0:1, seg])

            # ---- sign chain on DVE (gpsimd is ~20x too slow for elementwise,
            # ACT would add a second wait to every C op) -----------------------
            sgn = singles.tile([128, 2, HS], f32)
            for h in range(2):
                gt_t = work.tile([128, HS], f32)
                nc.vector.tensor_scalar(
                    out=gt_t[:], in0=w2_sb[:, h, :], scalar1=0.0, scalar2=None,
                    op0=mybir.AluOpType.is_gt,
                )
                lt_t = work.tile([128, HS], f32)
                nc.vector.tensor_scalar(
                    out=lt_t[:], in0=w2_sb[:, h, :], scalar1=0.0, scalar2=None,
                    op0=mybir.AluOpType.is_lt,
                )
                nc.vector.tensor_sub(sgn[:, h, :], gt_t[:], lt_t[:])

            # ---- A[p, t] = -eps * yT[p, t] ----------------------------------
            a_sb = singles.tile([128, T], f32)
            nc.vector.tensor_scalar_mul(a_sb[:], yt_sb[:], -EPS)

            # ---- C segments: C[h, seg] = sign(W2_h)[:, seg] * norm[seg] -----
            for t4 in range(4):
                seg = slice(t4 * SEG, (t4 + 1) * SEG)
                for h in range(2):
                    nc.vector.tensor_mul(
                        c_sb[:, h, seg], sgn[:, h, seg], bnorm_ps[:, seg]
                    )

            # ---- main loop: pert row-tiles, j-block 0 first -----------------
            last_tsp = None
            for b in range(NJ):
                jlo, jhi = b * SUB, (b + 1) * SUB
                t = 0
                for g_chunk in chunk_plan:
                    stage = stagep.tile([128, G, SUB], f32)
                    t0 = t
                    for g in range(g_chunk):
                        last_tsp = nc.vector.tensor_scalar_mul(
                            out=stage[:, g, :],
                            in0=c_sb[:, t % 2, jlo:jhi],
                            scalar1=a_sb[:, t : t + 1],
                        )
                        t += 1
                    nc.sync.dma_start(
                        out=pert_out[t0:t, :, jlo:jhi].rearrange("t p j -> p t j"),
                        in_=stage[:, 0:g_chunk, :],
                    )

            # ---- per-core partial row sums of W2 (for nn_output) ------------
            # Pinned after the main loop (explicit dep, since the scheduler
            # orders by readiness): the DVE tick gating this store then
            # transitively implies all earlier DMA completions, so its HWDGE
            # lane-reuse wait is provably dead and can be stripped.
            s_sb = singles.tile([128, 2], f32)
            for h in range(2):
                red = nc.vector.reduce_sum(
                    out=s_sb[:, h : h + 1],
                    in_=w2_sb[:, h, :],
                    axis=mybir.AxisListType.X,
                )
                tile.add_dep_helper(
                    red.ins, last_tsp.ins, sync=False,
                    reason="keep s-reduces after the pert main loop",
                )
            nc.sync.dma_start(out=s_out[:, :], in_=s_sb[:])
    return nc


def _strip_redundant_waits(nc):
    """Reduce every instruction to (at most) one semaphore wait when provably
    safe.  The walrus codegen in this toolchain supports only ONE sync wait
    per compute/DMA instruction, but Tile's semaphore assignment freely emits
    two (e.g. a data dep plus a DMA-lane-reuse dep that is transitively
    implied by the data dep).

    Soundness model: engines execute their instruction streams in order and a
    compute instruction's own-engine semaphore increments fire at completion
    before the next same-engine instruction dispatches.  DMA-lane semaphores
    (DMAHW*/DMASW*) increment at *transfer* completion, so they contribute
    guarantees only through explicit waits, never through program order.

    We propagate, per instruction X, a map G[X]: sem -> value guaranteed to
    have been reached when X dispatches:
      - same-engine predecessor's guarantees + its waits + its own-engine
        increments (serial completion),
      - for each wait (S, v) where S is an engine-owned semaphore, the full
        guarantee set of the producer instruction that raised S to v.
    A wait (S, v) on X is dead if the guarantees from X's *other* waits plus
    program order already imply S >= v.
    """
    fn = nc.m.functions[0]

    insts = []
    for bb in fn.blocks:
        insts.extend(bb.instructions)

    def sem_waits(inst):
        si = getattr(inst, "sync_info", None)
        if not si or not si.on_wait:
            return []
        return [
            w
            for w in si.on_wait
            if w.sync_type == "semaphore" and "ge" in str(w.wait_mode)
        ]

    # Identify engine-owned semaphores: updated exclusively by instructions of
    # one engine AND not a DMA-completion semaphore (those fire out of band).
    upd_engines = {}
    for inst in insts:
        si = getattr(inst, "sync_info", None)
        if si:
            for u in si.on_update or []:
                upd_engines.setdefault(u.id, set()).add(str(inst.engine))
    engine_sem_owner = {}
    for inst in insts:
        si = getattr(inst, "sync_info", None)
        if si:
            for u in si.on_update or []:
                name = u.ant_name or ""
                if name.startswith(("DMAHW", "DMASW")):
                    continue
                if upd_engines.get(u.id) and len(upd_engines[u.id]) == 1:
                    engine_sem_owner[u.id] = next(iter(upd_engines[u.id]))

    # Forward pass: cumulative own-engine increments and guarantee maps.
    def merge(dst, src):
        for k, v in src.items():
            if dst.get(k, -1) < v:
                dst[k] = v

    cum = {}  # sem_id -> cumulative inc
    producer_g = {}  # sem_id -> list of (cum_value_after, guarantee map)
    g_complete_prev = {}  # engine -> guarantee map after its last instruction
    g_dispatch = [None] * len(insts)

    def closure_of_wait(sem_id, value):
        # A wait (S, v) guarantees S >= v, plus everything the producer that
        # raised S to v was itself guaranteed.  For engine sems the producer's
        # completion-time guarantees apply; for DMA sems the transfer fires
        # after the DMACopy's dispatch, so its dispatch-time guarantees (incl.
        # its own waits) apply.
        out = {sem_id: value}
        hist = producer_g.get(sem_id, [])
        for cum_v, g in hist:
            if cum_v >= value:
                merge(out, g)
                break
        return out

    n_drop = 0
    for idx, inst in enumerate(insts):
        eng = str(inst.engine)
        g = dict(g_complete_prev.get(eng, {}))
        waits = sem_waits(inst)
        # prune: keep the minimal wait set (greedy: try each wait as the sole
        # keeper and check whether it implies all the others)
        si = getattr(inst, "sync_info", None)
        if si is not None and si.on_wait and len(si.on_wait) > 1 and waits:
            all_waits = list(si.on_wait)
            best = None
            for keeper in all_waits:
                ctx = dict(g)
                merge(ctx, closure_of_wait(keeper.id, keeper.wait_value or 0))
                if all(
                    (w is keeper)
                    or (ctx.get(w.id, -1) >= (w.wait_value or 0))
                    for w in all_waits
                ):
                    best = keeper
                    break
            if best is not None:
                n_drop += len(all_waits) - 1
                si.on_wait = [best]
                waits = [best]
            else:
                # keep all; compile may still reject, but don't guess
                pass
        # fold this instruction's (kept) waits into its dispatch guarantees
        for w in waits:
            merge(g, closure_of_wait(w.id, w.wait_value or 0))
        g_dispatch[idx] = g
        # completion: own-engine sem increments become guaranteed for the
        # next same-engine instruction; DMA-sem increments fire later (at
        # transfer completion) but still carry this dispatch's guarantees.
        g_after = dict(g)
        if si:
            for u in si.on_update or []:
                name = u.ant_name or ""
                if engine_sem_owner.get(u.id) == eng:
                    cum[u.id] = cum.get(u.id, 0) + (u.update_value or 0)
                    g_after[u.id] = max(g_after.get(u.id, -1), cum[u.id])
                    producer_g.setdefault(u.id, []).append((cum[u.id], g_after))
                elif name.startswith(("DMAHW", "DMASW")):
                    cum[u.id] = cum.get(u.id, 0) + (u.update_value or 0)
                    producer_g.setdefault(u.id, []).append((cum[u.id], dict(g)))
        g_complete_prev[eng] = g_after
    return n_drop


def _get_nc():
    global _NC_CACHE
    if _NC_CACHE is None:
        nc = _build_kernel()
        _strip_redundant_waits(nc)
        _NC_CACHE = nc
    return _NC_CACHE


def _run(inputs, trace=False):
    y = np.asarray(inputs["y"])
    W1 = np.asarray(inputs["W1"], dtype=np.float32)
    W2 = np.asarray(inputs["W2"], dtype=np.float32)
    bias2 = np.asarray(inputs["bias2"], dtype=np.float32)

    # yT[p, t] = y.flat[t*128 + p]  (so column t is the per-partition scalar
    # vector for row-tile t of the flattened (b, i) dim)
    yT = np.ascontiguousarray(y.astype(np.float32).reshape(T, 128).T)

    in_maps = []
    for c in range(NCORES):
        in_maps.append(
            {
                "w1s": np.ascontiguousarray(W1[c * HS : (c + 1) * HS, :]),
                "w2s": np.ascontiguousarray(W2[:, c * HS : (c + 1) * HS]),
                "yt": yT,
            }
        )

    nc = _get_nc()
    bres = run_bass_kernel_spmd(nc, in_maps, core_ids=list(range(NCORES)), trace=trace)
    res = bres.results

    pert = np.empty((B, O, H), dtype=np.float32)
    s_total = np.zeros(O, dtype=np.float32)
    for c in range(NCORES):
        pert[:, :, c * HS : (c + 1) * HS] = res[c]["pert_out"].reshape(B, O, HS)
        # s_out[p, h] = sum_j W2s[h*128+p, j]  ->  index i = h*128 + p
        s_total += res[c]["s_out"].T.reshape(O)

    nn_row = (np.float32(H) * s_total + bias2).astype(np.float32)
    nn_output = np.broadcast_to(nn_row, (B, O)).copy()
    return (nn_output, pert), bres


def kernel(**inputs):
    (nn_output, pert), _ = _run(inputs, trace=False)
    return nn_output, pert
